# revision 1
# baseline (speedup 1.0000x reference)
"""Trainium2 Bass kernel for nn_Attention_32049045963483 (sparse_attention).

Math collapse (verified vs reference at ~3e-6 rel err):
  - qkv 1x1 conv folds into the 11x11/stride-8 down-convs:
      conv(W1 @ f, wq) == conv(f, w_eff),  w_eff[oc,d] = sum_ic wq[oc,ic] W1[ic,d]
  - nearest-neighbor 64x upsample of the [64,64] score map + softmax over the
    upsampled axis == softmax of the low-res map; with row index i -> i//64 = x,
    every output row depends only on x.
  - v enters only through 64-wide block sums:  vbar[c,J] = sum_y v[c,J,y]
      = Wv @ fbar,  fbar[d,J] = sum_y f[d,J,y]   (v never materializes)
  - out[c,x,y] = (sum_J e[J,x] * vbar[c,J]) / (64 * sum_J e[J,x]),
      e[J,I] = exp(scale * dots[I,J])  -- broadcast along y.

Sharding: head-parallel over 8 cores. Core i computes global channels
8i..8i+7 (head i): conv out-channel slices of wq/wk, v-row slice of w_qkv.
Each core reads full f (the down-convs mix all 64 input channels).

Conv structure: factorized two-stage form so the heavy matmuls stream with
free dim >= 256, where float32r runs at full rate (plain fp32 is 1/4):
  stage 1: s[(ky,oc), r, ox] = sum_d w_eff[d,(ky,oc)]@kx . fpad[d, r, 8ox+kx]
           accumulated over kx, in two r-chunks (B: rows 34..66 first --
           matches DMA arrival order -- then A: rows 0..33)
  stage 2: q_low[oc,(oy,ox)]  = sum_ky s[(ky,oc), 8oy+ky, ox]
           via identity-slice selection stationaries; q and k accumulate in
           separate base-0 PSUM tiles so dots needs no partition rebase.
"""

import numpy as np

N_CORES = 8
SCALE = 8.0 ** -0.5  # dim_head ** -0.5

# packed [64, *] weight tensor columns: [wqr | w1q | wkr | w1k | wvt | bq | bk]
C_WQR = 0
C_W1Q = 968
C_WKR = 1032
C_W1K = 2000
C_WVT = 2064
C_BQ = 2072
C_BK = 2073
C_TOT = 2080

_CACHE = {}

LAST_RESULTS = None  # BassKernelResults of the most recent run (for test harness)


def _dep(after, before, sync=False):
    from concourse.tile import add_dep_helper

    a = getattr(after, "ins", after)
    b = getattr(before, "ins", before)
    add_dep_helper(a, b, sync=sync, reason="pin order")


def _build_nc():
    from contextlib import ExitStack

    import concourse.bacc as bacc
    import concourse.mybir as mybir
    import concourse.tile as tile

    f32 = mybir.dt.float32
    f32r = mybir.dt.float32r
    bf16 = mybir.dt.bfloat16
    X = mybir.AxisListType.X
    AF = mybir.ActivationFunctionType

    # Bacc (not raw Bass): its compile() splits >1-wait sync via event
    # semaphores -- hardware allows only one sync wait per instruction.
    nc = bacc.Bacc("TRN2", target_bir_lowering=False)

    f_d = nc.dram_tensor("f", [64, 68 * 68], f32r, kind="ExternalInput")
    wp_d = nc.dram_tensor("wp", [64, 2064], mybir.dt.float16, kind="ExternalInput")
    w2_d = nc.dram_tensor("w2", [64, 16], f32, kind="ExternalInput")
    ws_d = nc.dram_tensor("ws", [88, 88], f32r, kind="ExternalInput")
    out_d = nc.dram_tensor("out", [8, 4096], f32, kind="ExternalOutput")

    with tile.TileContext(nc) as tc:
        with ExitStack() as ctx:
            sb = ctx.enter_context(tc.tile_pool(name="sb", bufs=1))
            ps = ctx.enter_context(tc.tile_pool(name="ps", bufs=1, space="PSUM"))

            fpad = sb.tile([64, 68 * 68], f32r)
            wp_t = sb.tile([64, 2064], mybir.dt.float16)
            w2_t = sb.tile([64, 16], f32)
            ws_t = sb.tile([88, 88], f32r)
            wmq_t = sb.tile([64, 968], f32r)
            wmk_t = sb.tile([64, 968], f32r)
            sq_t = sb.tile([88, 536], f32r)
            sk_t = sb.tile([88, 536], f32r)
            q_t = sb.tile([8, 64], f32)
            k_t = sb.tile([8, 64], f32)
            e_t = sb.tile([64, 64], f32)
            fbar_t = sb.tile([64, 64], f32)
            vaug_t = sb.tile([64, 9], f32)
            rs_t = sb.tile([64, 1], f32)
            olT_t = sb.tile([64, 8], f32)
            T_t = sb.tile([64, 8 * 64], f32)
            scr_t = sb.tile([1, 1], f32)
            scr2_t = sb.tile([1, 1], f32)

            # --- input DMAs interleaved across both HWDGE rings in priority
            # order: conv weights, f rows 0..33 (A-chunks), f rows 34..67, ws
            fp3 = fpad.rearrange("p (r c) -> p r c", c=68)
            # Strict arrival priority at full 2-ring bandwidth: each f piece's
            # trigger waits on the previous priority class's completion on the
            # OTHER ring (same-ring order is implicit), so wp lands first,
            # then f rows 0..33 (A-chunks), then rows 34..67.
            d_wpA = nc.sync.dma_start(out=wp_t[:, 0:C_WKR], in_=wp_d[:, 0:C_WKR])
            d_wpB = nc.scalar.dma_start(out=wp_t[:, C_WKR:2064], in_=wp_d[:, C_WKR:2064])
            d_fA1 = nc.sync.dma_start(out=fpad[:, 0:1156], in_=f_d[:, 0:1156])
            d_fA2 = nc.scalar.dma_start(out=fpad[:, 1156:2312], in_=f_d[:, 1156:2312])
            d_fB1 = nc.sync.dma_start(out=fpad[:, 2312:3468], in_=f_d[:, 2312:3468])
            d_fB2 = nc.scalar.dma_start(out=fpad[:, 3468:4624], in_=f_d[:, 3468:4624])
            d_ws = nc.sync.dma_start(out=ws_t, in_=ws_d[:])
            d_w2 = nc.sync.dma_start(out=w2_t, in_=w2_d[:])

            # preload ACT function tables during the DMA wait (after the ACT
            # ring's DMA triggers). Exp first, Gelu LAST so the gelu set is
            # resident for the real GELUs; the exp reload hides behind dots.
            nc.vector.memset(scr_t, 0.0)
            nc.vector.memset(vaug_t[:, 8:9], 64.0)
            de = nc.scalar.activation(out=scr2_t, in_=scr_t, func=AF.Exp)
            dg = nc.scalar.activation(out=scr2_t, in_=scr_t, func=AF.Gelu)
            _dep(de, d_wpB)
            _dep(de, d_fB2)
            _dep(dg, de)

            wqr4 = wp_t[:, C_WQR:C_W1Q].rearrange(
                "p (kx ky oc) -> p kx ky oc", ky=11, oc=8
            )
            wkr4 = wp_t[:, C_WKR:C_W1K].rearrange(
                "p (kx ky oc) -> p kx ky oc", ky=11, oc=8
            )
            w1q = wp_t[:, C_W1Q:C_WKR]
            w1k = wp_t[:, C_W1K:C_WVT]
            wvt_v = w2_t[:, 0:8]
            bq_v = w2_t[0:8, 8:9]
            bk_v = w2_t[0:8, 9:10]

            # --- compose conv weights: w_eff[d,(kx,ky,oc)], f32r big-free MMs
            psq = ps.tile([64, 11 * 128], f32, tag="A")
            psk = ps.tile([64, 11 * 128], f32, tag="B")
            psq4 = psq.rearrange("p (kx pad) -> p kx pad", pad=128)
            psk4 = psk.rearrange("p (kx pad) -> p kx pad", pad=128)

            def compose(ps4, w1, wr4):
                for x0, x1 in ((0, 4), (4, 8), (8, 11)):
                    nc.tensor.matmul(
                        ps4[:, x0:x1, 0:88], w1, wr4[:, x0:x1],
                        start=True, stop=True,
                    )

            compose(psq4, w1q, wqr4)
            nc.vector.tensor_copy(out=wmq_t, in_=psq4[:, :, 0:88])
            compose(psk4, w1k, wkr4)
            nc.vector.tensor_copy(out=wmk_t, in_=psk4[:, :, 0:88])

            # --- stage 1: per conv, 11 kx accumulate; free = (r-chunk, ox)
            def s1(pst, wm, sl_r):
                out = []
                for kx in range(11):
                    out.append(nc.tensor.matmul(
                        pst, wm[:, kx * 88 : kx * 88 + 88],
                        fp3[:, sl_r, kx : kx + 57 : 8],
                        start=(kx == 0), stop=(kx == 10),
                    ))
                return out[0]

            slAr, slBr = slice(0, 34), slice(34, 67)

            # fbar sub-reduces interleave into DVE gaps (J in groups of 16);
            # parts 0,1 need only f rows 2..33 which arrive first
            def fbar_part(j):
                return nc.vector.reduce_sum(
                    out=fbar_t[:, 16 * j : 16 * (j + 1)],
                    in_=fp3[:, 2 + 16 * j : 18 + 16 * j, 2:66].bitcast(f32),
                    axis=X,
                )

            gateA = nc.tensor.ldweights(weights=fpad[:, 138:139].bitcast(bf16))
            gateA2 = nc.tensor.ldweights(weights=fpad[:, 1160:1161].bitcast(bf16))
            ps_qA = ps.tile([88, 272], f32, tag="C")
            ps_kA = ps.tile([88, 272], f32, tag="D")
            qa = s1(ps_qA, wmq_t, slAr)
            _dep(qa, gateA)
            _dep(qa, gateA2)
            fb0 = fbar_part(0)
            fb1 = fbar_part(1)
            cast_qA = nc.vector.tensor_copy(out=sq_t[:, 0:272], in_=ps_qA)
            ka = s1(ps_kA, wmk_t, slAr)
            cast_kA = nc.vector.tensor_copy(out=sk_t[:, 0:272], in_=ps_kA)

            gateB = nc.tensor.ldweights(weights=fpad[:, 2316:2317].bitcast(bf16))
            gateB2 = nc.tensor.ldweights(weights=fpad[:, 3473:3474].bitcast(bf16))
            ps_qB = ps.tile([88, 264], f32, tag="A")
            ps_kB = ps.tile([88, 264], f32, tag="B")
            qb = s1(ps_qB, wmq_t, slBr)
            _dep(qb, gateB)
            _dep(qb, gateB2)
            fb2 = fbar_part(2)
            fb3 = fbar_part(3)
            cast_qB = nc.vector.tensor_copy(out=sq_t[:, 272:536], in_=ps_qB)
            kb = s1(ps_kB, wmk_t, slBr)
            cast_kB = nc.vector.tensor_copy(out=sk_t[:, 272:536], in_=ps_kB)

            # --- stage 2: k then q, separate base-0 PSUM accumulators
            sq3 = sq_t.rearrange("p (rr ox) -> p rr ox", ox=8)
            sk3 = sk_t.rearrange("p (rr ox) -> p rr ox", ox=8)
            # q group first: its cast lands earlier, so PE streams stage-2-q
            # while the k-conv's last PSUM cast is still finishing on DVE
            psc_k = ps.tile([8, 64], f32, tag="D")
            psc_q = ps.tile([8, 64], f32, tag="C")
            for ky in range(11):
                nc.tensor.matmul(
                    psc_q, ws_t[:, ky * 8 : ky * 8 + 8],
                    sq3[:, ky : ky + 57 : 8, :],
                    start=(ky == 0), stop=(ky == 10),
                )
            nc.scalar.activation(
                out=q_t, in_=psc_q, func=AF.Gelu, bias=bq_v, scale=1.0
            )
            for ky in range(11):
                nc.tensor.matmul(
                    psc_k, ws_t[:, ky * 8 : ky * 8 + 8],
                    sk3[:, ky : ky + 57 : 8, :],
                    start=(ky == 0), stop=(ky == 10),
                )
            nc.scalar.activation(
                out=k_t, in_=psc_k, func=AF.Gelu, bias=bk_v, scale=1.0
            )

            # --- vbar path (fbar parts already reduced during stage 1)
            gate_v = nc.tensor.ldweights(weights=fbar_t[:, 0:1].bitcast(bf16))
            gate_v2 = nc.tensor.ldweights(weights=fbar_t[:, 63:64].bitcast(bf16))
            psv = ps.tile([64, 8], f32, tag="A")
            vmm = nc.tensor.matmul(
                psv, fbar_t, wvt_v, start=True, stop=True
            )
            _dep(vmm, gate_v)
            _dep(vmm, gate_v2)
            nc.scalar.copy(out=vaug_t[:, 0:8], in_=psv)

            # --- dots_T[J,I] = sum_c k[c,J] q[c,I];  e = exp(scale * dots_T)
            gate2 = nc.tensor.ldweights(weights=k_t[:, 0:1].bitcast(bf16))
            psd = ps.tile([64, 64], f32, tag="B")
            dmm = nc.tensor.matmul(psd, k_t, q_t, start=True, stop=True)
            _dep(dmm, gate2)
            nc.scalar.activation(out=e_t, in_=psd, func=AF.Exp, scale=SCALE)

            # --- out_u[I, 0:8] = sum_J e[J,I] vbar[J,c]; col 8 = 64*sum_J e
            gate_o = nc.tensor.ldweights(weights=e_t[:, 0:1].bitcast(bf16))
            pso = ps.tile([64, 9], f32, tag="A")
            omm = nc.tensor.matmul(pso, e_t, vaug_t, start=True, stop=True)
            _dep(omm, gate_o)
            nc.vector.reciprocal(out=rs_t, in_=pso[:, 8:9])
            nc.vector.tensor_scalar_mul(olT_t, pso[:, 0:8], rs_t)

            # --- broadcast along y: single DVE copy with stride-0 read on y
            import concourse.bass as bass
            T3 = T_t.rearrange("p (c y) -> p c y", y=64)
            ola = olT_t[:]
            ol_b = bass.AP(
                tensor=ola.tensor, offset=ola.offset,
                ap=[list(ola.ap[0]), list(ola.ap[1]), [0, 64]],
            )
            nc.vector.tensor_copy(out=T3, in_=ol_b)

            # --- store: out[c, x, y] <- T[x, c, y]
            out_ap = out_d[:].rearrange("c (x y) -> c x y", y=64).transpose([1, 0, 2])
            nc.sync.dma_start(out=out_ap, in_=T3)

    nc.finalize()
    return nc


def _get_nc():
    if "nc" not in _CACHE:
        _CACHE["nc"] = _build_nc()
    return _CACHE["nc"]


_WSEL = np.eye(88, dtype=np.float32)


def kernel(**inputs):
    global LAST_RESULTS
    from concourse.bass_utils import run_bass_kernel_spmd

    f = np.ascontiguousarray(inputs["f"], np.float32)
    w_qkv = np.ascontiguousarray(inputs["w_qkv"], np.float32)[:, :, 0, 0]  # [192,64]
    wq = np.ascontiguousarray(inputs["wq"], np.float32)
    wk = np.ascontiguousarray(inputs["wk"], np.float32)
    bq = np.ascontiguousarray(inputs["bq"], np.float32)
    bk = np.ascontiguousarray(inputs["bk"], np.float32)

    f2 = np.zeros((64, 68, 68), np.float32)
    f2[:, 2:66, 2:66] = f[0]
    f2 = f2.reshape(64, 68 * 68)

    in_maps = []
    for i in range(N_CORES):
        sl = slice(8 * i, 8 * i + 8)
        wp = np.zeros((64, 2064), np.float16)
        # [oc,ic,ky,kx] slice -> [ic,kx,ky,oc]
        wp[:, C_WQR:C_W1Q] = wq[sl].transpose(1, 3, 2, 0).reshape(64, 968)
        wp[:, C_W1Q:C_WKR] = w_qkv[0:64]
        wp[:, C_WKR:C_W1K] = wk[sl].transpose(1, 3, 2, 0).reshape(64, 968)
        wp[:, C_W1K:C_WVT] = w_qkv[64:128]
        w2 = np.zeros((64, 16), np.float32)
        w2[:, 0:8] = w_qkv[128 + 8 * i : 136 + 8 * i].T
        w2[0:8, 8] = bq[sl]
        w2[0:8, 9] = bk[sl]
        in_maps.append({"f": f2, "wp": wp, "w2": w2, "ws": _WSEL})

    nc = _get_nc()
    res = run_bass_kernel_spmd(nc, in_maps, core_ids=list(range(N_CORES)))
    LAST_RESULTS = res
    out = np.concatenate([r["out"] for r in res.results], axis=0)  # [64, 4096]
    return out.reshape(1, 64, 64, 64)

